# revision 9
# baseline (speedup 1.0000x reference)
"""Trainium2 Bass kernel for AttributionCentroidTracker — gather-first, v17.

Reference computation (B=512, V=32768, C=16):
    Wg[b, v]   = W_eff[b, v, labels[b]]
    attr[b, v] = |sparse_vector[b, v] * Wg[b, v]|
    sums[c, v] = segment_sum(attr, labels)       # [C, V]
    out        = where(present, where(init, lerp(cent, mean), mean), cent)

Only 1/16 of W_eff (channel labels[b] per sample) enters the result: the host
stages W transposed per core to [B, C, VSH] bf16 and the device fetches just
the 512 selected rows.

v17 over v16: the row gather uses the MAINLINE SWDGE indirect DMA
(indirect_dma_start / InstDMACopy, runtime int32 offset table, one offset
per output partition, flat element offsets via axis=1) instead of the
mlp-library dma_gather. The mlp GPSIMD library image (~4 MiB / ~12 us of
Q7 copy that gated every gather until ~19 us) is gone entirely — gathers
now start right after the Pool preamble and overlap the sv stream; the
whole kernel becomes SDMA-throughput-bound.

  - attr = |Wg*sv| in place: DVE tensor_tensor mult (bf16 2x) +
    int16-bitcast bitwise_and 0x7FFF (tensor_scalar 4x).
  - segment-sum on TensorE: per 512-column chunk j, accumulating matmuls
    (a_c-scaled one-hot lhsT [128,32], cols 16..31 zero) at PSUM partition
    32*(j%4) via tile_position; 4 chunks share each [128,512] PSUM tile.
  - b_c*centroids seeded into PSUM by 8 small bf16 matmuls before the
    slices accumulate; ScalarE evacuates the two finished PSUM tiles; the
    4 output DMAs alternate between the SP and ACT HWDGE rings.
a_c/b_c fold count-normalization, EMA, and init/present selection exactly;
the host computes only labels/initialized-derived [16]/[512]-sized tables.
"""

import os
import sys

import numpy as np

if "/opt/trn_rl_repo" not in sys.path:
    sys.path.insert(0, "/opt/trn_rl_repo")

B, V, C = 512, 32768, 16
NCORES = 8
VSH = V // NCORES            # 4096
P = 128
BG = B // P                  # 4 (b = r*128 + p)
NSPLIT = 4                   # gather slices along B
GPH = BG // NSPLIT           # 1 row-group per slice
VC = 512                     # psum chunk columns
NCH = VSH // VC              # 8 chunks
CPT = 4                      # chunks per psum tile (col groups)
NPT = NCH // CPT             # 2 psum tiles
STEPS_PER_EPOCH = 1000
MOMENTUM = 1.0 - 2.0 / (STEPS_PER_EPOCH + 1)

_CACHE = {}

last_exec_time_ns = None
last_results = None


def _build_nc():
    import concourse.bacc as bacc
    import concourse.tile as tile
    from concourse import mybir
    from concourse.bass import IndirectOffsetOnAxis

    f32 = mybir.dt.float32
    bf16 = mybir.dt.bfloat16
    i32 = mybir.dt.int32
    nc = bacc.Bacc("TRN2", target_bir_lowering=False, debug=False)

    wt = nc.dram_tensor("wt", [B * C, VSH], bf16, kind="ExternalInput")
    sv = nc.dram_tensor("sv", [P, BG * VSH], bf16, kind="ExternalInput")
    idx = nc.dram_tensor("idx", [P, NSPLIT], i32, kind="ExternalInput")
    ohsa = nc.dram_tensor("ohsa", [P, BG * 32], bf16, kind="ExternalInput")
    centt = nc.dram_tensor("centt", [P, NPT * VC], bf16, kind="ExternalInput")
    bdg = nc.dram_tensor("bdg", [P, 32], bf16, kind="ExternalInput")
    out = nc.dram_tensor("out", [C, VSH], f32, kind="ExternalOutput")

    with tile.TileContext(nc) as tc:
        with (
            tc.tile_pool(name="const", bufs=1) as cpool,
            tc.tile_pool(name="wg", bufs=NSPLIT) as wgpool,
            tc.tile_pool(name="psum", bufs=1, space="PSUM") as ppool,
        ):
            # idx first on the SP ring (gathers wait only on it); other
            # small tables on the ACT ring.
            idx_sb = cpool.tile([P, NSPLIT], dtype=i32)
            nc.sync.dma_start(out=idx_sb[:], in_=idx.ap())
            ohsa_sb = cpool.tile([P, BG * 32], dtype=bf16)
            nc.scalar.dma_start(out=ohsa_sb[:], in_=ohsa.ap())
            bdg_sb = cpool.tile([P, 32], dtype=bf16)
            nc.scalar.dma_start(out=bdg_sb[:], in_=bdg.ap())

            # Bulk sv stream on the SP ring (one DMA per slice so slice h's
            # completion unblocks DVE h early), then cent.
            # sv host layout [p, r, v] (b = r*128 + p); slice h = row-group h.
            sv_sb = cpool.tile([P, BG * VSH], dtype=bf16)
            HB = GPH * VSH                       # elems per slice per partition
            for h in range(NSPLIT):
                nc.sync.dma_start(
                    out=sv_sb[:][:, h * HB : (h + 1) * HB],
                    in_=sv.ap()[:, h * HB : (h + 1) * HB],
                )
            centt_sb = cpool.tile([P, NPT * VC], dtype=bf16)
            nc.scalar.dma_start(out=centt_sb[:], in_=centt.ap())

            # Gathers: slice h covers rows b = h*128 .. h*128+127
            # (wg[h][p, :] = Wg[b = h*128 + p] = wt row idx[p, h] / VSH).
            # Mainline SWDGE indirect DMA: one int32 flat-element offset per
            # output partition, axis=1 so the offset coefficient is 1.
            wgs = []
            for h in range(NSPLIT):
                wgt = wgpool.tile([P, GPH * VSH], dtype=bf16, tag="wg")
                wg3 = wgt[:].rearrange("p (g v) -> p g v", g=GPH)
                nc.gpsimd.indirect_dma_start(
                    out=wgt[:],
                    out_offset=None,
                    in_=wt.ap(),
                    in_offset=IndirectOffsetOnAxis(
                        ap=idx_sb[:, h : h + 1], axis=1
                    ),
                )
                wgs.append((wgt, wg3))

            pss = [
                ppool.tile([P, VC], dtype=f32, name=f"ps{t}", tag=f"ps{t}")
                for t in range(NPT)
            ]
            out_sb = cpool.tile([P, NPT * VC], dtype=f32)
            ob4 = out_sb[:].rearrange("p (t v) -> p t v", t=NPT)
            ct4 = centt_sb[:].rearrange("p (t v) -> p t v", t=NPT)
            outr = out.ap().rearrange(
                "c (t j v) -> j c t v", t=NPT, j=CPT, v=VC
            )

            # Seed psum[32jj+c, :] = b_c * cent (8 small K=32 matmuls; the
            # chunk groups then accumulate across all slices, so the Tile
            # group checker is skipped — has_written is per element).
            for j in range(NCH):
                t, jj = j // CPT, j % CPT
                base = 32 * jj
                nc.tensor.matmul(
                    out=pss[t][base : base + 32, :],
                    lhsT=bdg_sb[base : base + 32, :],
                    rhs=ct4[base : base + 32, t],
                    start=True,
                    stop=False,
                    tile_position=(base, base),
                    skip_group_check=True,
                )

            for h in range(NSPLIT):
                wgt, wg3 = wgs[h]
                # attr = |Wg * sv| (flat contiguous views, in place)
                nc.vector.tensor_tensor(
                    out=wgt[:],
                    in0=wgt[:],
                    in1=sv_sb[:][:, h * HB : (h + 1) * HB],
                    op=mybir.AluOpType.mult,
                )
                nc.vector.tensor_scalar(
                    out=wgt[:].bitcast(mybir.dt.int16),
                    in0=wgt[:].bitcast(mybir.dt.int16),
                    scalar1=0x7FFF,
                    scalar2=None,
                    op0=mybir.AluOpType.bitwise_and,
                )
                for j in range(NCH):
                    t, jj = j // CPT, j % CPT
                    base = 32 * jj
                    for g in range(GPH):
                        r = h * GPH + g
                        nc.tensor.matmul(
                            out=pss[t][base : base + 32, :],
                            lhsT=ohsa_sb[:, r * 32 : (r + 1) * 32],
                            rhs=wg3[:, g, j * VC : (j + 1) * VC],
                            start=False,
                            stop=(r == BG - 1),
                            tile_position=(0, base),
                            skip_group_check=True,
                        )

            for t in range(NPT):
                nc.scalar.activation(
                    ob4[:, t], pss[t][:], mybir.ActivationFunctionType.Copy
                )
            for jj in range(CPT):
                eng = nc.sync if jj % 2 == 0 else nc.scalar
                eng.dma_start(
                    out=outr[jj],
                    in_=ob4[32 * jj : 32 * jj + C],
                )

    nc.finalize()
    return nc


def _get_nc():
    if "nc" not in _CACHE:
        _CACHE["nc"] = _build_nc()
    return _CACHE["nc"]


def _stage_inputs(sparse_vector, W_eff, labels, centroids, initialized):
    import ml_dtypes

    bf16 = ml_dtypes.bfloat16
    sv = np.asarray(sparse_vector, dtype=np.float32)
    w = np.asarray(W_eff, dtype=np.float32)
    lab = np.asarray(labels).astype(np.int64)
    cent = np.asarray(centroids, dtype=np.float32)
    init = np.asarray(initialized).astype(bool)

    # Host-side label-derived constants (tiny [16]) — program stays generic.
    oh = lab[:, None] == np.arange(C)[None, :]
    counts = oh.sum(axis=0).astype(np.float64)
    present = counts > 0
    safe = np.maximum(counts, 1.0)
    a = np.where(present, np.where(init, (1.0 - MOMENTUM) / safe, 1.0 / safe), 0.0)
    b = np.where(present, np.where(init, MOMENTUM, 0.0), 1.0)

    # diag(b_c) padded to 32 and replicated per col group.
    bd32 = np.zeros((32, 32), np.float32)
    bd32[:C, :C] = np.diag(b)
    bdg = np.tile(bd32, (CPT, 1)).astype(bf16)  # [128, 32]

    # Gather offsets: idx[p, h] = flat element offset of wt row for sample
    # b = h*128 + p (row b*C+lab[b], times VSH elements per row).
    row_idx = (np.arange(B, dtype=np.int64) * C + lab) * VSH
    idx_arr = np.ascontiguousarray(
        row_idx.reshape(NSPLIT, P).T.astype(np.int32)
    )  # [128, NSPLIT]

    # a_c-scaled one-hots, M padded to 32 (cols 16..31 zero so the full
    # PSUM col-group is written): ohsa[p, r*32+c] = a_c iff labels[r*128+p]==c
    lab2 = lab.reshape(BG, P)
    ohsa = np.zeros((P, BG * 32), np.float32)
    for r in range(BG):
        ohsa[:, r * 32 : r * 32 + C] = (
            lab2[r][:, None] == np.arange(C)[None, :]
        ) * a[None, :]
    ohsa = ohsa.astype(bf16)

    # sv in [p, r, v] layout (b = r*128 + p), bf16.
    sv_t = np.ascontiguousarray(
        sv.astype(bf16).reshape(BG, P, V).transpose(1, 0, 2)
    )  # [128, BG, V]

    w16 = w.astype(bf16)

    in_maps = []
    for i in range(NCORES):
        s = i * VSH
        svs = np.ascontiguousarray(sv_t[:, :, s : s + VSH])  # [p, r, vsh]
        # cent packed [128, (t, v)]: partition 32*jj+c -> cent[c, (t*4+jj)*VC+v]
        cslice = cent[:, s : s + VSH].reshape(C, NCH, VC)
        centt = np.zeros((P, NPT, VC), np.float32)  # cast to bf16 below
        for j in range(NCH):
            t, jj = j // CPT, j % CPT
            centt[32 * jj : 32 * jj + C, t, :] = cslice[:, j, :]
        in_maps.append(
            {
                "wt": np.ascontiguousarray(
                    w16[:, s : s + VSH, :].transpose(0, 2, 1)
                ).reshape(B * C, VSH),
                "sv": svs.reshape(P, BG * VSH),
                "idx": idx_arr,
                "ohsa": ohsa,
                "centt": centt.reshape(P, NPT * VC).astype(bf16),
                "bdg": bdg,
            }
        )
    return in_maps


def kernel(sparse_vector, W_eff, labels, centroids, initialized):
    global last_exec_time_ns, last_results
    from concourse.bass_utils import run_bass_kernel_spmd

    in_maps = _stage_inputs(sparse_vector, W_eff, labels, centroids, initialized)
    nc = _get_nc()
    res = run_bass_kernel_spmd(nc, in_maps, core_ids=list(range(NCORES)))
    last_exec_time_ns = res.exec_time_ns
    last_results = res
    return np.concatenate([res.results[i]["out"] for i in range(NCORES)], axis=1)


# revision 12
# speedup vs baseline: 1.0949x; 1.0949x over previous
"""Trainium2 Bass kernel for AttributionCentroidTracker — gather-first, v17.

Reference computation (B=512, V=32768, C=16):
    Wg[b, v]   = W_eff[b, v, labels[b]]
    attr[b, v] = |sparse_vector[b, v] * Wg[b, v]|
    sums[c, v] = segment_sum(attr, labels)       # [C, V]
    out        = where(present, where(init, lerp(cent, mean), mean), cent)

Only 1/16 of W_eff (channel labels[b] per sample) enters the result: the host
stages W transposed per core to [B, C, VSH] bf16 and the device fetches just
the 512 selected rows.

v17 over v16: the row gather uses the MAINLINE SWDGE indirect DMA
(indirect_dma_start / InstDMACopy, runtime int32 offset table, one offset
per output partition, flat element offsets via axis=1) instead of the
mlp-library dma_gather. The mlp GPSIMD library image (~4 MiB / ~12 us of
Q7 copy that gated every gather until ~19 us) is gone entirely — gathers
now start right after the Pool preamble and overlap the sv stream; the
whole kernel becomes SDMA-throughput-bound.

  - attr = |Wg*sv| in place: DVE tensor_tensor mult (bf16 2x) +
    int16-bitcast bitwise_and 0x7FFF (tensor_scalar 4x).
  - segment-sum on TensorE: per 512-column chunk j, accumulating matmuls
    (a_c-scaled one-hot lhsT [128,32], cols 16..31 zero) at PSUM partition
    32*(j%4) via tile_position; 4 chunks share each [128,512] PSUM tile.
  - b_c*centroids seeded into PSUM by 8 small bf16 matmuls before the
    slices accumulate; ScalarE evacuates the two finished PSUM tiles; the
    4 output DMAs alternate between the SP and ACT HWDGE rings.
a_c/b_c fold count-normalization, EMA, and init/present selection exactly;
the host computes only labels/initialized-derived [16]/[512]-sized tables.
"""

import os
import sys

import numpy as np

if "/opt/trn_rl_repo" not in sys.path:
    sys.path.insert(0, "/opt/trn_rl_repo")

B, V, C = 512, 32768, 16
NCORES = 8
VSH = V // NCORES            # 4096
P = 128
BG = B // P                  # 4 (b = r*128 + p)
NSPLIT = 4                   # gather slices along B
GPH = BG // NSPLIT           # 1 row-group per slice
VC = 512                     # psum chunk columns
NCH = VSH // VC              # 8 chunks
CPT = 4                      # chunks per psum tile (col groups)
NPT = NCH // CPT             # 2 psum tiles
STEPS_PER_EPOCH = 1000
MOMENTUM = 1.0 - 2.0 / (STEPS_PER_EPOCH + 1)

_CACHE = {}

last_exec_time_ns = None
last_results = None


def _build_nc():
    import concourse.bacc as bacc
    import concourse.tile as tile
    from concourse import mybir
    from concourse.bass import IndirectOffsetOnAxis

    f32 = mybir.dt.float32
    bf16 = mybir.dt.bfloat16
    i32 = mybir.dt.int32
    nc = bacc.Bacc(
        "TRN2", target_bir_lowering=False, debug=False, num_swdge_queues=4
    )

    wt = nc.dram_tensor("wt", [B * C, VSH], bf16, kind="ExternalInput")
    sv = nc.dram_tensor("sv", [P, BG * VSH], bf16, kind="ExternalInput")
    idx = nc.dram_tensor("idx", [P, NSPLIT], i32, kind="ExternalInput")
    ohsa = nc.dram_tensor("ohsa", [P, BG * 32], bf16, kind="ExternalInput")
    centt = nc.dram_tensor("centt", [P, NPT * VC], bf16, kind="ExternalInput")
    bdg = nc.dram_tensor("bdg", [P, 32], bf16, kind="ExternalInput")
    out = nc.dram_tensor("out", [C, VSH], f32, kind="ExternalOutput")

    with tile.TileContext(nc) as tc:
        with (
            tc.tile_pool(name="const", bufs=1) as cpool,
            tc.tile_pool(name="wg", bufs=NSPLIT) as wgpool,
            tc.tile_pool(name="psum", bufs=1, space="PSUM") as ppool,
        ):
            # idx first on the ACT ring (tiny; gathers wait only on it);
            # other small tables follow it there. sv heads the SP ring so
            # its first packet flies as early as possible.
            idx_sb = cpool.tile([P, NSPLIT], dtype=i32)
            nc.scalar.dma_start(out=idx_sb[:], in_=idx.ap())

            # Bulk sv stream on the SP ring (one DMA per slice so slice h's
            # completion unblocks DVE h early), then cent.
            # sv host layout [p, r, v] (b = r*128 + p); slice h = row-group h.
            sv_sb = cpool.tile([P, BG * VSH], dtype=bf16)
            HB = GPH * VSH                       # elems per slice per partition
            for h in range(NSPLIT):
                nc.sync.dma_start(
                    out=sv_sb[:][:, h * HB : (h + 1) * HB],
                    in_=sv.ap()[:, h * HB : (h + 1) * HB],
                )
            ohsa_sb = cpool.tile([P, BG * 32], dtype=bf16)
            nc.scalar.dma_start(out=ohsa_sb[:], in_=ohsa.ap())
            bdg_sb = cpool.tile([P, 32], dtype=bf16)
            nc.scalar.dma_start(out=bdg_sb[:], in_=bdg.ap())
            centt_sb = cpool.tile([P, NPT * VC], dtype=bf16)
            nc.scalar.dma_start(out=centt_sb[:], in_=centt.ap())

            # Gathers: slice h covers rows b = h*128 .. h*128+127
            # (wg[h][p, :] = Wg[b = h*128 + p] = wt row idx[p, h] / VSH).
            # Mainline SWDGE indirect DMA: one int32 flat-element offset per
            # output partition, axis=1 so the offset coefficient is 1.
            wgs = []
            for h in range(NSPLIT):
                wgt = wgpool.tile([P, GPH * VSH], dtype=bf16, tag="wg")
                wg3 = wgt[:].rearrange("p (g v) -> p g v", g=GPH)
                gi = nc.gpsimd.indirect_dma_start(
                    out=wgt[:],
                    out_offset=None,
                    in_=wt.ap(),
                    in_offset=IndirectOffsetOnAxis(
                        ap=idx_sb[:, h : h + 1], axis=1
                    ),
                )
                # spread the four gathers across the four SWDGE queues so
                # their SDMA drains can proceed in parallel
                gi.queue = f"qPoolDynamic{h or ''}"
                wgs.append((wgt, wg3))

            pss = [
                ppool.tile([P, VC], dtype=f32, name=f"ps{t}", tag=f"ps{t}")
                for t in range(NPT)
            ]
            out_sb = cpool.tile([P, NPT * VC], dtype=f32)
            ob4 = out_sb[:].rearrange("p (t v) -> p t v", t=NPT)
            ct4 = centt_sb[:].rearrange("p (t v) -> p t v", t=NPT)
            outr = out.ap().rearrange(
                "c (t j v) -> j c t v", t=NPT, j=CPT, v=VC
            )

            # Seed psum[32jj+c, :] = b_c * cent (8 small K=32 matmuls; the
            # chunk groups then accumulate across all slices, so the Tile
            # group checker is skipped — has_written is per element).
            for j in range(NCH):
                t, jj = j // CPT, j % CPT
                base = 32 * jj
                nc.tensor.matmul(
                    out=pss[t][base : base + 32, :],
                    lhsT=bdg_sb[base : base + 32, :],
                    rhs=ct4[base : base + 32, t],
                    start=True,
                    stop=False,
                    tile_position=(base, base),
                    skip_group_check=True,
                )

            for h in range(NSPLIT):
                wgt, wg3 = wgs[h]
                # attr = |Wg * sv| (flat contiguous views, in place)
                nc.vector.tensor_tensor(
                    out=wgt[:],
                    in0=wgt[:],
                    in1=sv_sb[:][:, h * HB : (h + 1) * HB],
                    op=mybir.AluOpType.mult,
                )
                nc.vector.tensor_scalar(
                    out=wgt[:].bitcast(mybir.dt.int16),
                    in0=wgt[:].bitcast(mybir.dt.int16),
                    scalar1=0x7FFF,
                    scalar2=None,
                    op0=mybir.AluOpType.bitwise_and,
                )
                for j in range(NCH):
                    t, jj = j // CPT, j % CPT
                    base = 32 * jj
                    for g in range(GPH):
                        r = h * GPH + g
                        nc.tensor.matmul(
                            out=pss[t][base : base + 32, :],
                            lhsT=ohsa_sb[:, r * 32 : (r + 1) * 32],
                            rhs=wg3[:, g, j * VC : (j + 1) * VC],
                            start=False,
                            stop=(r == BG - 1),
                            tile_position=(0, base),
                            skip_group_check=True,
                        )

            for t in range(NPT):
                nc.scalar.activation(
                    ob4[:, t], pss[t][:], mybir.ActivationFunctionType.Copy
                )
            for jj in range(CPT):
                eng = nc.sync if jj % 2 == 0 else nc.scalar
                eng.dma_start(
                    out=outr[jj],
                    in_=ob4[32 * jj : 32 * jj + C],
                )

    nc.finalize()
    return nc


def _get_nc():
    if "nc" not in _CACHE:
        _CACHE["nc"] = _build_nc()
    return _CACHE["nc"]


def _stage_inputs(sparse_vector, W_eff, labels, centroids, initialized):
    import ml_dtypes

    bf16 = ml_dtypes.bfloat16
    sv = np.asarray(sparse_vector, dtype=np.float32)
    w = np.asarray(W_eff, dtype=np.float32)
    lab = np.asarray(labels).astype(np.int64)
    cent = np.asarray(centroids, dtype=np.float32)
    init = np.asarray(initialized).astype(bool)

    # Host-side label-derived constants (tiny [16]) — program stays generic.
    oh = lab[:, None] == np.arange(C)[None, :]
    counts = oh.sum(axis=0).astype(np.float64)
    present = counts > 0
    safe = np.maximum(counts, 1.0)
    a = np.where(present, np.where(init, (1.0 - MOMENTUM) / safe, 1.0 / safe), 0.0)
    b = np.where(present, np.where(init, MOMENTUM, 0.0), 1.0)

    # diag(b_c) padded to 32 and replicated per col group.
    bd32 = np.zeros((32, 32), np.float32)
    bd32[:C, :C] = np.diag(b)
    bdg = np.tile(bd32, (CPT, 1)).astype(bf16)  # [128, 32]

    # Gather offsets: idx[p, h] = flat element offset of wt row for sample
    # b = h*128 + p (row b*C+lab[b], times VSH elements per row).
    row_idx = (np.arange(B, dtype=np.int64) * C + lab) * VSH
    idx_arr = np.ascontiguousarray(
        row_idx.reshape(NSPLIT, P).T.astype(np.int32)
    )  # [128, NSPLIT]

    # a_c-scaled one-hots, M padded to 32 (cols 16..31 zero so the full
    # PSUM col-group is written): ohsa[p, r*32+c] = a_c iff labels[r*128+p]==c
    lab2 = lab.reshape(BG, P)
    ohsa = np.zeros((P, BG * 32), np.float32)
    for r in range(BG):
        ohsa[:, r * 32 : r * 32 + C] = (
            lab2[r][:, None] == np.arange(C)[None, :]
        ) * a[None, :]
    ohsa = ohsa.astype(bf16)

    # sv in [p, r, v] layout (b = r*128 + p), bf16.
    sv_t = np.ascontiguousarray(
        sv.astype(bf16).reshape(BG, P, V).transpose(1, 0, 2)
    )  # [128, BG, V]

    w16 = w.astype(bf16)

    in_maps = []
    for i in range(NCORES):
        s = i * VSH
        svs = np.ascontiguousarray(sv_t[:, :, s : s + VSH])  # [p, r, vsh]
        # cent packed [128, (t, v)]: partition 32*jj+c -> cent[c, (t*4+jj)*VC+v]
        cslice = cent[:, s : s + VSH].reshape(C, NCH, VC)
        centt = np.zeros((P, NPT, VC), np.float32)  # cast to bf16 below
        for j in range(NCH):
            t, jj = j // CPT, j % CPT
            centt[32 * jj : 32 * jj + C, t, :] = cslice[:, j, :]
        in_maps.append(
            {
                "wt": np.ascontiguousarray(
                    w16[:, s : s + VSH, :].transpose(0, 2, 1)
                ).reshape(B * C, VSH),
                "sv": svs.reshape(P, BG * VSH),
                "idx": idx_arr,
                "ohsa": ohsa,
                "centt": centt.reshape(P, NPT * VC).astype(bf16),
                "bdg": bdg,
            }
        )
    return in_maps


def kernel(sparse_vector, W_eff, labels, centroids, initialized):
    global last_exec_time_ns, last_results
    from concourse.bass_utils import run_bass_kernel_spmd

    in_maps = _stage_inputs(sparse_vector, W_eff, labels, centroids, initialized)
    nc = _get_nc()
    res = run_bass_kernel_spmd(nc, in_maps, core_ids=list(range(NCORES)))
    last_exec_time_ns = res.exec_time_ns
    last_results = res
    return np.concatenate([res.results[i]["out"] for i in range(NCORES)], axis=1)
